# revision 4
# baseline (speedup 1.0000x reference)
"""Trainium2 Bass kernel for ConditionCrossAttention2D.

Reference computation (per batch item b, with n = H*W spatial positions):
    q = Wq @ cond + bq            # [Ck, n] -> used as q[n, Ck]
    k = Wk @ feat + bk            # [Ck, n]
    v = Wv @ feat + bv            # [C, n]
    energy[i, j] = sum_ck q[ck, i] * k[ck, j]
    attn = softmax_j(energy)
    out[c, i] = sum_j v[c, j] * attn[i, j]
    result = gamma * out + feat

Sharding: 8 cores = (batch b in 0..3) x (query-half h in 0..1). Each core
computes the full [2048 x 4096] attention for its query half — no
cross-core communication.

Per-core layout choices:
  - energy is computed TRANSPOSED: e_T[j, i] (keys on partitions). The
    exp'd tile attnT[j, i] is then directly the stationary operand (lhsT)
    of the PV matmul out[i, c] = sum_j attnT[j, i] * vT[j, c]. No
    transposes in the inner loop.
  - softmax denominators come for free from an appended ones-column in
    vT (vT[j, 256] = 1), so out_psum[i, 256] = sum_j exp(energy[i, j]).
  - softmax max-subtraction is skipped: energies here are O(1) (weights
    are 0.02-scaled), exp is computed in fp32 — mathematically identical
    to the max-shifted softmax.
  - matmul inputs are cast to bf16 (fp32 PSUM accumulation), softmax
    statistics and the output path stay fp32.
"""

import os
from contextlib import ExitStack

import numpy as np

import concourse.bass as bass
import concourse.tile as tile
from concourse import mybir
from concourse.bass_utils import run_bass_kernel_spmd
from concourse.masks import make_identity

B, C, CK, H, W = 4, 256, 32, 64, 64
N = H * W            # 4096 spatial positions
NCORES = 8
NL = N // 2          # 2048 queries per core
P = 128
NJT = N // P         # 32 key tiles
NIT = NL // P        # 16 query tiles per core
GJ = 8               # key tiles per group
NG = NJT // GJ       # 4 groups
F32 = mybir.dt.float32
BF16 = mybir.dt.bfloat16

LAST_EXEC_TIME_NS = None
LAST_TRACE = None

ts = bass.ts


def _emit(tc, ctx):
    nc = tc.nc

    feat_d = nc.declare_dram_parameter("feat", [C, N], F32, isOutput=False)
    cond_d = nc.declare_dram_parameter("cond", [C, NL], F32, isOutput=False)
    fres_d = nc.declare_dram_parameter("fres", [C, NL], F32, isOutput=False)
    wq_d = nc.declare_dram_parameter("Wq", [CK, C], F32, isOutput=False)
    wk_d = nc.declare_dram_parameter("Wk", [CK, C], F32, isOutput=False)
    wv_d = nc.declare_dram_parameter("Wv", [C, C], F32, isOutput=False)
    bq_d = nc.declare_dram_parameter("bq", [CK], F32, isOutput=False)
    bk_d = nc.declare_dram_parameter("bk", [CK], F32, isOutput=False)
    bv_d = nc.declare_dram_parameter("bv", [C], F32, isOutput=False)
    gam_d = nc.declare_dram_parameter("gamma", [1], F32, isOutput=False)
    out_d = nc.declare_dram_parameter("out", [C, NL], F32, isOutput=True)

    def bcast_ap(handle, parts, free):
        ap = handle[:]
        return bass.AP(tensor=ap.tensor, offset=ap.offset, ap=[[0, parts], [1, free]])

    consts = ctx.enter_context(tc.tile_pool(name="consts", bufs=1))
    persist = ctx.enter_context(tc.tile_pool(name="persist", bufs=1))

    ident = consts.tile([P, P], F32)
    make_identity(nc, ident)

    # Transposed weights (bf16): wq_t[p, ct, ck] = Wq[ck, ct*128+p]
    wq_t = consts.tile([P, 2, CK], BF16)
    wk_t = consts.tile([P, 2, CK], BF16)
    # wv_t[p, ct, c] = Wv[c, ct*128+p]; column 256 stays 0
    wv_t = consts.tile([P, 2, C + 1], BF16)
    nc.vector.memset(wv_t[:], 0.0)
    # bv broadcast across partitions; column 256 = 1.0 (ones column of vT)
    bv_b = consts.tile([P, C + 1], F32)
    nc.vector.memset(bv_b[:], 1.0)
    nc.gpsimd.dma_start(out=bv_b[:, 0:C], in_=bcast_ap(bv_d, P, C))
    bq_c = consts.tile([CK, 1], F32)
    nc.sync.dma_start(out=bq_c[:], in_=bq_d[:][:, None])
    bk_c = consts.tile([CK, 1], F32)
    nc.sync.dma_start(out=bk_c[:], in_=bk_d[:][:, None])
    gam = consts.tile([P, 1], F32)
    nc.gpsimd.dma_start(out=gam[:], in_=bcast_ap(gam_d, P, 1))

    # Residual features for this core's query half: [p, ct, i]
    feat_res = persist.tile([P, 2, NL], F32)
    for ct in range(2):
        nc.sync.dma_start(out=feat_res[:, ct, :], in_=fres_d[ts(ct, P), :])

    # Projection outputs (persist across phases)
    q_sb = persist.tile([CK, NL], BF16)        # q[ck, i]
    k_sb = persist.tile([CK, N], BF16)         # k[ck, j]
    vT_sb = persist.tile([P, NJT, C + 1], BF16)  # vT[j%128, jt, c] (+ones col)

    # ---- Phase A: load + cast inputs, project q/k/vT ----
    with tc.tile_pool(name="loads", bufs=1) as loads, \
         tc.tile_pool(name="castbuf", bufs=3) as castbuf, \
         tc.tile_pool(name="abuf", bufs=1) as abuf, \
         tc.tile_pool(name="psA", bufs=2, space="PSUM") as psA:

        wq_raw = loads.tile([CK, C], F32)
        nc.sync.dma_start(out=wq_raw[:], in_=wq_d[:, :])
        wk_raw = loads.tile([CK, C], F32)
        nc.sync.dma_start(out=wk_raw[:], in_=wk_d[:, :])
        wv_raw = loads.tile([P, 2, C], F32)
        for cb in range(2):
            nc.sync.dma_start(out=wv_raw[:, cb, :], in_=wv_d[ts(cb, P), :])

        feat_bf = abuf.tile([P, 2, N], BF16)
        cond_bf = abuf.tile([P, 2, NL], BF16)
        for ct in range(2):
            for ch in range(N // 512):
                tmp = castbuf.tile([P, 512], F32, tag="cast")
                nc.sync.dma_start(out=tmp[:], in_=feat_d[ts(ct, P), ts(ch, 512)])
                nc.vector.tensor_copy(feat_bf[:, ct, ts(ch, 512)], tmp[:])
            for ch in range(NL // 512):
                tmp = castbuf.tile([P, 512], F32, tag="cast")
                nc.sync.dma_start(out=tmp[:], in_=cond_d[ts(ct, P), ts(ch, 512)])
                nc.vector.tensor_copy(cond_bf[:, ct, ts(ch, 512)], tmp[:])

        # Weight transposes via PE
        for ct in range(2):
            ps = psA.tile([P, CK], F32, tag="proj")
            nc.tensor.transpose(ps[:], wq_raw[:, ts(ct, P)], ident[0:CK, 0:CK])
            nc.vector.tensor_copy(wq_t[:, ct, :], ps[:])
            ps = psA.tile([P, CK], F32, tag="proj")
            nc.tensor.transpose(ps[:], wk_raw[:, ts(ct, P)], ident[0:CK, 0:CK])
            nc.vector.tensor_copy(wk_t[:, ct, :], ps[:])
        for cb in range(2):
            for ct in range(2):
                ps = psA.tile([P, P], F32, tag="proj")
                nc.tensor.transpose(ps[:], wv_raw[:, cb, ts(ct, P)], ident[:])
                nc.vector.tensor_copy(wv_t[:, ct, ts(cb, P)], ps[:])

        # q[ck, i] = sum_c Wq[ck, c] cond[c, i]  (+bq on the PSUM->SBUF copy)
        q_ps = psA.tile([CK, NL], F32, tag="proj")
        for icc in range(NL // 512):
            for ct in range(2):
                nc.tensor.matmul(
                    q_ps[:, ts(icc, 512)], wq_t[:, ct, :],
                    cond_bf[:, ct, ts(icc, 512)],
                    start=(ct == 0), stop=(ct == 1))
        nc.vector.tensor_scalar(q_sb[:], q_ps[:], bq_c[:], None,
                                op0=mybir.AluOpType.add)

        # k[ck, j], in two column halves
        for kh in range(2):
            k_ps = psA.tile([CK, NL], F32, tag="proj")
            for ncc in range(NL // 512):
                for ct in range(2):
                    nc.tensor.matmul(
                        k_ps[:, ts(ncc, 512)], wk_t[:, ct, :],
                        feat_bf[:, ct, kh * NL + ncc * 512:kh * NL + (ncc + 1) * 512],
                        start=(ct == 0), stop=(ct == 1))
            nc.vector.tensor_scalar(k_sb[:, ts(kh, NL)], k_ps[:], bk_c[:], None,
                                    op0=mybir.AluOpType.add)

        # vT[j, c] = sum_cf feat[cf, j] Wv[c, cf]  (+bv, +ones column)
        for jt in range(NJT):
            v_ps = psA.tile([P, C + 1], F32, tag="proj")
            for ct in range(2):
                nc.tensor.matmul(
                    v_ps[:], feat_bf[:, ct, ts(jt, P)], wv_t[:, ct, :],
                    start=(ct == 0), stop=(ct == 1))
            nc.vector.tensor_tensor(vT_sb[:, jt, :], v_ps[:], bv_b[:],
                                    op=mybir.AluOpType.add)

    # ---- Phase B: energy -> exp -> PV, grouped over key tiles ----
    out_acc = persist.tile([P, NIT, C + 1], F32)

    with tc.tile_pool(name="attn", bufs=2) as attnp, \
         tc.tile_pool(name="eps", bufs=2, space="PSUM") as epsp, \
         tc.tile_pool(name="pvps", bufs=2, space="PSUM") as pvpsp:

        for g in range(NG):
            attnT = attnp.tile([P, GJ, NL], BF16)
            for jl in range(GJ):
                jt = g * GJ + jl
                for ih in range(2):
                    e_ps = epsp.tile([P, 1024], F32)
                    for ic2 in range(2):
                        nc.tensor.matmul(
                            e_ps[:, ts(ic2, 512)],
                            k_sb[:, ts(jt, P)],
                            q_sb[:, ih * 1024 + ic2 * 512:ih * 1024 + (ic2 + 1) * 512],
                            start=True, stop=True)
                    nc.scalar.activation(
                        attnT[:, jl, ts(ih, 1024)], e_ps[:],
                        mybir.ActivationFunctionType.Exp)
            for it in range(NIT):
                pv = pvpsp.tile([P, C + 1], F32)
                for jl in range(GJ):
                    nc.tensor.matmul(
                        pv[:], attnT[:, jl, ts(it, P)], vT_sb[:, g * GJ + jl, :],
                        start=(jl == 0), stop=(jl == GJ - 1))
                if g == 0:
                    nc.vector.tensor_copy(out_acc[:, it, :], pv[:])
                else:
                    nc.vector.tensor_tensor(out_acc[:, it, :], pv[:],
                                            out_acc[:, it, :],
                                            op=mybir.AluOpType.add)

    # ---- Phase C: normalize, transpose back to [c, i], residual, store ----
    with tc.tile_pool(name="fin", bufs=3) as finp, \
         tc.tile_pool(name="stage", bufs=1) as stagep, \
         tc.tile_pool(name="tpps", bufs=2, space="PSUM") as tppsp:

        out_stage = stagep.tile([P, 2, NL], F32)
        for it in range(NIT):
            rcp = finp.tile([P, 1], F32, tag="rcp")
            nc.vector.reciprocal(rcp[:], out_acc[:, it, C:C + 1])
            on = finp.tile([P, C], F32, tag="on")
            nc.vector.tensor_scalar(on[:], out_acc[:, it, 0:C], rcp[:], None,
                                    op0=mybir.AluOpType.mult)
            for ct in range(2):
                tp = tppsp.tile([P, P], F32)
                nc.tensor.transpose(tp[:], on[:, ts(ct, P)], ident[:])
                nc.vector.scalar_tensor_tensor(
                    out_stage[:, ct, ts(it, P)], tp[:], gam[:],
                    feat_res[:, ct, ts(it, P)],
                    op0=mybir.AluOpType.mult, op1=mybir.AluOpType.add)
        for ct in range(2):
            nc.sync.dma_start(out=out_d[ts(ct, P), :], in_=out_stage[:, ct, :])


def _split_ctrl_waits(nc, cap=1):
    """Walrus in this image allows only ONE sync-wait command per
    instruction; Tile emits several on phase-boundary instructions (and one
    per live semaphore on the kernel-tail drain). Splitting the excess waits
    onto preceding same-engine NoOps is semantically identical (engine
    sequencers execute in order, so waiting on A then B == waiting on both)."""
    for fn in nc.m.functions:
        for bb in fn.blocks:
            insts = bb.instructions
            out = []
            changed = False
            for ins in insts:
                si = ins.sync_info
                if si is not None and si.on_wait and len(si.on_wait) > cap:
                    waits = list(si.on_wait)
                    for i, w in enumerate(waits[:-cap]):
                        nop = mybir.InstNoOp(
                            name=f"{ins.name}-w{i}",
                            engine=ins.engine,
                            ins=[], outs=[],
                            sync_info=mybir.SyncInfo(on_wait=[w], on_update=[]),
                        )
                        if hasattr(nc, "register_instruction"):
                            nc.register_instruction(nop, overwrite=True)
                        out.append(nop)
                    ins.sync_info = mybir.SyncInfo(
                        on_wait=waits[-cap:], on_update=list(si.on_update))
                    changed = True
                out.append(ins)
            if changed:
                insts[:] = out


def build_nc():
    nc = bass.Bass()
    with tile.TileContext(nc) as tc, ExitStack() as ctx:
        _emit(tc, ctx)
    _split_ctrl_waits(nc)
    return nc


def make_in_maps(features, conditions, Wq, bq, Wk, bk, Wv, bv, gamma):
    feat = np.ascontiguousarray(np.asarray(features, np.float32).reshape(B, C, N))
    cond = np.ascontiguousarray(np.asarray(conditions, np.float32).reshape(B, C, N))
    wq = np.ascontiguousarray(np.asarray(Wq, np.float32))
    wk = np.ascontiguousarray(np.asarray(Wk, np.float32))
    wv = np.ascontiguousarray(np.asarray(Wv, np.float32))
    bq_ = np.ascontiguousarray(np.asarray(bq, np.float32))
    bk_ = np.ascontiguousarray(np.asarray(bk, np.float32))
    bv_ = np.ascontiguousarray(np.asarray(bv, np.float32))
    gam_ = np.ascontiguousarray(np.asarray(gamma, np.float32).reshape(1))
    in_maps = []
    for core in range(NCORES):
        b, h = divmod(core, 2)
        n0 = h * NL
        in_maps.append({
            "feat": feat[b],
            "cond": np.ascontiguousarray(cond[b][:, n0:n0 + NL]),
            "fres": np.ascontiguousarray(feat[b][:, n0:n0 + NL]),
            "Wq": wq, "Wk": wk, "Wv": wv,
            "bq": bq_, "bk": bk_, "bv": bv_, "gamma": gam_,
        })
    return in_maps


def kernel(features, conditions, Wq, bq, Wk, bk, Wv, bv, gamma):
    global LAST_EXEC_TIME_NS, LAST_TRACE
    in_maps = make_in_maps(features, conditions, Wq, bq, Wk, bk, Wv, bv, gamma)
    nc = build_nc()
    trace = os.environ.get("BASS_KERNEL_TRACE", "0") == "1"
    res = run_bass_kernel_spmd(nc, in_maps, list(range(NCORES)), trace=trace)
    LAST_EXEC_TIME_NS = res.exec_time_ns
    LAST_TRACE = res.instructions_and_trace
    out = np.empty((B, C, N), np.float32)
    for core in range(NCORES):
        b, h = divmod(core, 2)
        out[b][:, h * NL:(h + 1) * NL] = res.results[core]["out"]
    return out.reshape(B, C, H, W)


# revision 7
# speedup vs baseline: 1.1609x; 1.1609x over previous
"""Trainium2 Bass kernel for ConditionCrossAttention2D.

Reference computation (per batch item b, with n = H*W spatial positions):
    q = Wq @ cond + bq            # [Ck, n] -> used as q[n, Ck]
    k = Wk @ feat + bk            # [Ck, n]
    v = Wv @ feat + bv            # [C, n]
    energy[i, j] = sum_ck q[ck, i] * k[ck, j]
    attn = softmax_j(energy)
    out[c, i] = sum_j v[c, j] * attn[i, j]
    result = gamma * out + feat

Sharding: 8 cores = (batch b in 0..3) x (query-half h in 0..1). Each core
computes the full [2048 x 4096] attention for its query half — no
cross-core communication.

Per-core layout choices:
  - energy is computed TRANSPOSED: e_T[j, i] (keys on partitions). The
    exp'd tile attnT[j, i] is then directly the stationary operand (lhsT)
    of the PV matmul out[i, c] = sum_j attnT[j, i] * vT[j, c]. No
    transposes in the inner loop.
  - softmax denominators come for free from an appended ones-column in
    vT (vT[j, 256] = 1), so out_psum[i, 256] = sum_j exp(energy[i, j]).
  - softmax max-subtraction is skipped: energies here are O(1) (weights
    are 0.02-scaled), exp is computed in fp32 — mathematically identical
    to the max-shifted softmax.
  - inputs are cast fp32->bf16 inside the load DMAs (SWDGE casting
    path), so no on-chip cast pass is needed.
  - the K=32 energy matmuls are packed 2x into disjoint PE row-groups
    via tile_position, with q/k replicated to partitions 32..63.
  - energy/PV matmuls use bf16 operands (fp32 PSUM accumulation);
    softmax statistics and the output path stay fp32.
"""

import os
from contextlib import ExitStack

import numpy as np

import concourse.bass as bass
import concourse.tile as tile
from concourse import mybir
from concourse.bass_utils import run_bass_kernel_spmd
from concourse.masks import make_identity

B, C, CK, H, W = 4, 256, 32, 64, 64
N = H * W            # 4096 spatial positions
NCORES = 8
NL = N // 2          # 2048 queries per core
P = 128
NJT = N // P         # 32 key tiles
NIT = NL // P        # 16 query tiles per core
GJ = 8               # key tiles per group
NG = NJT // GJ       # 4 groups
F32 = mybir.dt.float32
F32R = mybir.dt.float32r
BF16 = mybir.dt.bfloat16

LAST_EXEC_TIME_NS = None
LAST_TRACE = None

ts = bass.ts


def _emit(tc, ctx):
    nc = tc.nc

    feat_d = nc.declare_dram_parameter("feat", [C, N], F32, isOutput=False)
    cond_d = nc.declare_dram_parameter("cond", [C, NL], F32, isOutput=False)
    fres_d = nc.declare_dram_parameter("fres", [C, NL], F32, isOutput=False)
    wq_d = nc.declare_dram_parameter("Wq", [CK, C], F32, isOutput=False)
    wk_d = nc.declare_dram_parameter("Wk", [CK, C], F32, isOutput=False)
    wv_d = nc.declare_dram_parameter("Wv", [C, C], F32, isOutput=False)
    bq_d = nc.declare_dram_parameter("bq", [CK], F32, isOutput=False)
    bk_d = nc.declare_dram_parameter("bk", [CK], F32, isOutput=False)
    bv_d = nc.declare_dram_parameter("bv", [C], F32, isOutput=False)
    gam_d = nc.declare_dram_parameter("gamma", [1], F32, isOutput=False)
    out_d = nc.declare_dram_parameter("out", [C, NL], F32, isOutput=True)

    def bcast_ap(handle, parts, free):
        ap = handle[:]
        return bass.AP(tensor=ap.tensor, offset=ap.offset, ap=[[0, parts], [1, free]])

    consts = ctx.enter_context(tc.tile_pool(name="consts", bufs=1))
    persist = ctx.enter_context(tc.tile_pool(name="persist", bufs=1))

    ident = consts.tile([P, P], F32)
    make_identity(nc, ident)

    # Transposed weights (fp32, used as float32r): wq_t[p, ct, 32r+ck] =
    # Wq[ck, ct*128+p] for replica r in {0,1} (feeds the 2x-packed energy).
    wq_t = consts.tile([P, 2, 2 * CK], BF16)
    wk_t = consts.tile([P, 2, 2 * CK], BF16)
    # wv_t[p, ct, c] = Wv[c, ct*128+p]; column 256 stays 0
    wv_t = consts.tile([P, 2, C + 1], BF16)
    nc.vector.memset(wv_t[:], 0.0)
    # bv broadcast across partitions; column 256 = 1.0 (ones column of vT)
    bv_b = consts.tile([P, C + 1], F32)
    nc.vector.memset(bv_b[:], 1.0)
    nc.gpsimd.dma_start(out=bv_b[:, 0:C], in_=bcast_ap(bv_d, P, C))
    # per-partition bias columns, replicated for partitions 32..63
    bq_c = consts.tile([2 * CK, 1], F32)
    nc.sync.dma_start(out=bq_c[0:CK, :], in_=bq_d[:][:, None])
    nc.sync.dma_start(out=bq_c[CK:2 * CK, :], in_=bq_d[:][:, None])
    bk_c = consts.tile([2 * CK, 1], F32)
    nc.sync.dma_start(out=bk_c[0:CK, :], in_=bk_d[:][:, None])
    nc.sync.dma_start(out=bk_c[CK:2 * CK, :], in_=bk_d[:][:, None])
    gam = consts.tile([P, 1], F32)
    nc.gpsimd.dma_start(out=gam[:], in_=bcast_ap(gam_d, P, 1))

    # Residual features for this core's query half: [p, ct, i]
    feat_res = persist.tile([P, 2, NL], F32)
    for ct in range(2):
        nc.sync.dma_start(out=feat_res[:, ct, :], in_=fres_d[ts(ct, P), :])

    # Projection outputs (persist across phases); partitions 32..63 hold a
    # replica of partitions 0..31 (for the 2x-packed energy matmuls).
    q_rep = persist.tile([P, NL], BF16)         # q[ck, i] x2
    k_rep = persist.tile([P, N], BF16)          # k[ck, j] x2
    vT_sb = persist.tile([P, NJT, C + 1], BF16)  # vT[j%128, jt, c] (+ones col)
    out_acc = persist.tile([P, NIT, C + 1], F32)

    # ---- Phase A: load fp32 inputs, project q/k/vT (float32r matmuls) ----
    with tc.tile_pool(name="loads", bufs=1) as loads, \
         tc.tile_pool(name="psA", bufs=2, space="PSUM") as psA:

        wq_raw = loads.tile([CK, C], F32)
        nc.sync.dma_start(out=wq_raw[:], in_=wq_d[:, :])
        wk_raw = loads.tile([CK, C], F32)
        nc.sync.dma_start(out=wk_raw[:], in_=wk_d[:, :])
        wv_raw = loads.tile([P, 2, C], F32)
        for cb in range(2):
            nc.sync.dma_start(out=wv_raw[:, cb, :], in_=wv_d[ts(cb, P), :])

        # bf16 inputs via casting SWDGE DMAs, split by n-half so early
        # matmuls start sooner
        feat_sb = []
        for nh in range(2):
            t = loads.tile([P, 2, NL], BF16, tag=f"feat{nh}")
            for ct in range(2):
                nc.gpsimd.dma_start(out=t[:, ct, :],
                                    in_=feat_d[ts(ct, P), ts(nh, NL)])
            feat_sb.append(t)
        cond_sb = loads.tile([P, 2, NL], BF16)
        for ct in range(2):
            nc.gpsimd.dma_start(out=cond_sb[:, ct, :], in_=cond_d[ts(ct, P), :])

        # Weight transposes via PE; copy each psum twice to build replicas
        for ct in range(2):
            ps = psA.tile([P, CK], F32, tag="proj")
            nc.tensor.transpose(ps[:], wq_raw[:, ts(ct, P)], ident[0:CK, 0:CK])
            nc.vector.tensor_copy(wq_t[:, ct, 0:CK], ps[:])
            nc.vector.tensor_copy(wq_t[:, ct, CK:2 * CK], ps[:])
            ps = psA.tile([P, CK], F32, tag="proj")
            nc.tensor.transpose(ps[:], wk_raw[:, ts(ct, P)], ident[0:CK, 0:CK])
            nc.vector.tensor_copy(wk_t[:, ct, 0:CK], ps[:])
            nc.vector.tensor_copy(wk_t[:, ct, CK:2 * CK], ps[:])
        for cb in range(2):
            for ct in range(2):
                ps = psA.tile([P, P], F32, tag="proj")
                nc.tensor.transpose(ps[:], wv_raw[:, cb, ts(ct, P)], ident[:])
                nc.vector.tensor_copy(wv_t[:, ct, ts(cb, P)], ps[:])

        # q[ck, i] = sum_c Wq[ck, c] cond[c, i]  (+bq on the PSUM->SBUF copy)
        q_ps = psA.tile([P, NL], F32, tag="proj")
        for icc in range(NL // 512):
            for ct in range(2):
                nc.tensor.matmul(
                    q_ps[0:2 * CK, ts(icc, 512)], wq_t[:, ct, :],
                    cond_sb[:, ct, ts(icc, 512)],
                    start=(ct == 0), stop=(ct == 1))
        nc.vector.tensor_scalar(q_rep[0:2 * CK, :], q_ps[0:2 * CK, :],
                                bq_c[:], None, op0=mybir.AluOpType.add)

        # k[ck, j], per n-half
        for kh in range(2):
            k_ps = psA.tile([P, NL], F32, tag="proj")
            for ncc in range(NL // 512):
                for ct in range(2):
                    nc.tensor.matmul(
                        k_ps[0:2 * CK, ts(ncc, 512)], wk_t[:, ct, :],
                        feat_sb[kh][:, ct, ts(ncc, 512)],
                        start=(ct == 0), stop=(ct == 1))
            nc.vector.tensor_scalar(k_rep[0:2 * CK, ts(kh, NL)],
                                    k_ps[0:2 * CK, :], bk_c[:], None,
                                    op0=mybir.AluOpType.add)

        # vT[j, c] = sum_cf feat[cf, j] Wv[c, cf]  (+bv, +ones column)
        for jt in range(NJT):
            v_ps = psA.tile([P, C + 1], F32, tag="proj")
            nh, jl = divmod(jt, NJT // 2)
            for ct in range(2):
                nc.tensor.matmul(
                    v_ps[:], feat_sb[nh][:, ct, ts(jl, P)],
                    wv_t[:, ct, :],
                    start=(ct == 0), stop=(ct == 1))
            nc.vector.tensor_tensor(vT_sb[:, jt, :], v_ps[:], bv_b[:],
                                    op=mybir.AluOpType.add)

    # ---- Phase B: energy -> exp -> PV (grouped), finalize per query tile --
    with tc.tile_pool(name="attn", bufs=2) as attnp, \
         tc.tile_pool(name="fin", bufs=3) as finp, \
         tc.tile_pool(name="stage", bufs=2) as stagep, \
         tc.tile_pool(name="eps", bufs=2, space="PSUM") as epsp, \
         tc.tile_pool(name="pvps", bufs=2, space="PSUM") as pvpsp, \
         tc.tile_pool(name="tpps", bufs=2, space="PSUM") as tppsp:

        stage_tiles = {}

        def finalize(it):
            # out[i, c] /= out[i, 256]; transpose to [c, i]; residual+gamma
            rcp = finp.tile([P, 1], F32, tag="rcp")
            nc.vector.reciprocal(rcp[:], out_acc[:, it, C:C + 1])
            on = finp.tile([P, C], F32, tag="on")
            nc.vector.tensor_scalar(on[:], out_acc[:, it, 0:C], rcp[:], None,
                                    op0=mybir.AluOpType.mult)
            qt, sl = divmod(it, 4)
            if sl == 0:
                st_tile = stagep.tile([P, 2, 512], F32, tag="stage")
                stage_tiles[qt] = st_tile
            st = stage_tiles[qt]
            for ct in range(2):
                tp = tppsp.tile([P, P], F32)
                nc.tensor.transpose(tp[:], on[:, ts(ct, P)], ident[:])
                nc.vector.scalar_tensor_tensor(
                    st[:, ct, ts(sl, P)], tp[:], gam[:],
                    feat_res[:, ct, ts(it, P)],
                    op0=mybir.AluOpType.mult, op1=mybir.AluOpType.add)
            if sl == 3:
                for ct in range(2):
                    nc.sync.dma_start(out=out_d[ts(ct, P), ts(qt, 512)],
                                      in_=st[:, ct, :])

        for g in range(NG):
            attnT = attnp.tile([P, GJ, NL], BF16)
            for jl in range(GJ):
                jt = g * GJ + jl
                for ih in range(2):
                    e_ps = epsp.tile([P, 1024], F32)
                    # 2x-packed: replicas on partitions 0..31 / 32..63 feed
                    # disjoint PE row-groups, running concurrently
                    nc.tensor.matmul(
                        e_ps[:, 0:512],
                        k_rep[0:CK, ts(jt, P)],
                        q_rep[0:CK, ih * 1024:ih * 1024 + 512],
                        start=True, stop=True, tile_position=(0, 0))
                    nc.tensor.matmul(
                        e_ps[:, 512:1024],
                        k_rep[CK:2 * CK, ts(jt, P)],
                        q_rep[CK:2 * CK, ih * 1024 + 512:ih * 1024 + 1024],
                        start=True, stop=True, tile_position=(32, 0))
                    nc.scalar.activation(
                        attnT[:, jl, ts(ih, 1024)], e_ps[:],
                        mybir.ActivationFunctionType.Exp)
            for it in range(NIT):
                pv = pvpsp.tile([P, C + 1], F32)
                for jl in range(GJ):
                    nc.tensor.matmul(
                        pv[:], attnT[:, jl, ts(it, P)], vT_sb[:, g * GJ + jl, :],
                        start=(jl == 0), stop=(jl == GJ - 1))
                if g == 0:
                    nc.vector.tensor_copy(out_acc[:, it, :], pv[:])
                else:
                    nc.vector.tensor_tensor(out_acc[:, it, :], pv[:],
                                            out_acc[:, it, :],
                                            op=mybir.AluOpType.add)
                    if g == NG - 1:
                        finalize(it)


def _split_ctrl_waits(nc, cap=1):
    """Walrus in this image allows only ONE sync-wait command per
    instruction; Tile emits several on phase-boundary instructions (and one
    per live semaphore on the kernel-tail drain). Splitting the excess waits
    onto preceding same-engine NoOps is semantically identical (engine
    sequencers execute in order, so waiting on A then B == waiting on both)."""
    for fn in nc.m.functions:
        for bb in fn.blocks:
            insts = bb.instructions
            out = []
            changed = False
            for ins in insts:
                si = ins.sync_info
                if si is not None and si.on_wait and len(si.on_wait) > cap:
                    waits = list(si.on_wait)
                    for i, w in enumerate(waits[:-cap]):
                        nop = mybir.InstNoOp(
                            name=f"{ins.name}-w{i}",
                            engine=ins.engine,
                            ins=[], outs=[],
                            sync_info=mybir.SyncInfo(on_wait=[w], on_update=[]),
                        )
                        if hasattr(nc, "register_instruction"):
                            nc.register_instruction(nop, overwrite=True)
                        out.append(nop)
                    ins.sync_info = mybir.SyncInfo(
                        on_wait=waits[-cap:], on_update=list(si.on_update))
                    changed = True
                out.append(ins)
            if changed:
                insts[:] = out


def build_nc():
    nc = bass.Bass()
    with tile.TileContext(nc) as tc, ExitStack() as ctx:
        _emit(tc, ctx)
    _split_ctrl_waits(nc)
    return nc


def make_in_maps(features, conditions, Wq, bq, Wk, bk, Wv, bv, gamma):
    feat = np.ascontiguousarray(np.asarray(features, np.float32).reshape(B, C, N))
    cond = np.ascontiguousarray(np.asarray(conditions, np.float32).reshape(B, C, N))
    wq = np.ascontiguousarray(np.asarray(Wq, np.float32))
    wk = np.ascontiguousarray(np.asarray(Wk, np.float32))
    wv = np.ascontiguousarray(np.asarray(Wv, np.float32))
    bq_ = np.ascontiguousarray(np.asarray(bq, np.float32))
    bk_ = np.ascontiguousarray(np.asarray(bk, np.float32))
    bv_ = np.ascontiguousarray(np.asarray(bv, np.float32))
    gam_ = np.ascontiguousarray(np.asarray(gamma, np.float32).reshape(1))
    in_maps = []
    for core in range(NCORES):
        b, h = divmod(core, 2)
        n0 = h * NL
        in_maps.append({
            "feat": feat[b],
            "cond": np.ascontiguousarray(cond[b][:, n0:n0 + NL]),
            "fres": np.ascontiguousarray(feat[b][:, n0:n0 + NL]),
            "Wq": wq, "Wk": wk, "Wv": wv,
            "bq": bq_, "bk": bk_, "bv": bv_, "gamma": gam_,
        })
    return in_maps


def kernel(features, conditions, Wq, bq, Wk, bk, Wv, bv, gamma):
    global LAST_EXEC_TIME_NS, LAST_TRACE
    in_maps = make_in_maps(features, conditions, Wq, bq, Wk, bk, Wv, bv, gamma)
    nc = build_nc()
    trace = os.environ.get("BASS_KERNEL_TRACE", "0") == "1"
    res = run_bass_kernel_spmd(nc, in_maps, list(range(NCORES)), trace=trace)
    LAST_EXEC_TIME_NS = res.exec_time_ns
    LAST_TRACE = res.instructions_and_trace
    out = np.empty((B, C, N), np.float32)
    for core in range(NCORES):
        b, h = divmod(core, 2)
        out[b][:, h * NL:(h + 1) * NL] = res.results[core]["out"]
    return out.reshape(B, C, H, W)


# revision 8
# speedup vs baseline: 1.3886x; 1.1961x over previous
"""Trainium2 Bass kernel for ConditionCrossAttention2D.

Reference computation (per batch item b, with n = H*W spatial positions):
    q = Wq @ cond + bq            # [Ck, n] -> used as q[n, Ck]
    k = Wk @ feat + bk            # [Ck, n]
    v = Wv @ feat + bv            # [C, n]
    energy[i, j] = sum_ck q[ck, i] * k[ck, j]
    attn = softmax_j(energy)
    out[c, i] = sum_j v[c, j] * attn[i, j]
    result = gamma * out + feat

Sharding: 8 cores = (batch b in 0..3) x (query-half h in 0..1). Each core
computes the full [2048 x 4096] attention for its query half — no
cross-core communication.

Per-core layout choices:
  - energy is computed TRANSPOSED: e_T[j, i] (keys on partitions). The
    exp'd tile attnT[j, i] is then directly the stationary operand (lhsT)
    of the PV matmul out[i, c] = sum_j attnT[j, i] * vT[j, c]. No
    transposes in the inner loop.
  - softmax denominators come for free from an appended ones-column in
    vT (vT[j, 256] = 1), so out_psum[i, 256] = sum_j exp(energy[i, j]).
  - softmax max-subtraction is skipped: energies here are O(1) (weights
    are 0.02-scaled), exp is computed in fp32 — mathematically identical
    to the max-shifted softmax.
  - inputs are cast fp32->bf16 inside the load DMAs (SWDGE casting
    path), so no on-chip cast pass is needed.
  - the K=32 energy matmuls are packed 2x into disjoint PE row-groups
    via tile_position, with q/k replicated to partitions 32..63.
  - energy/PV matmuls use bf16 operands (fp32 PSUM accumulation);
    softmax statistics and the output path stay fp32.
"""

import os
from contextlib import ExitStack

import numpy as np

import concourse.bass as bass
import concourse.tile as tile
from concourse import mybir
from concourse.bass_utils import run_bass_kernel_spmd
from concourse.masks import make_identity

B, C, CK, H, W = 4, 256, 32, 64, 64
N = H * W            # 4096 spatial positions
NCORES = 8
NL = N // 2          # 2048 queries per core
P = 128
NJT = N // P         # 32 key tiles
NIT = NL // P        # 16 query tiles per core
GJ = 8               # key tiles per group
NG = NJT // GJ       # 4 groups
F32 = mybir.dt.float32
F32R = mybir.dt.float32r
BF16 = mybir.dt.bfloat16

LAST_EXEC_TIME_NS = None
LAST_TRACE = None

ts = bass.ts


def _emit(tc, ctx):
    nc = tc.nc

    feat_d = nc.declare_dram_parameter("feat", [C, N], F32, isOutput=False)
    cond_d = nc.declare_dram_parameter("cond", [C, NL], F32, isOutput=False)
    fres_d = nc.declare_dram_parameter("fres", [C, NL], F32, isOutput=False)
    wq_d = nc.declare_dram_parameter("Wq", [CK, C], F32, isOutput=False)
    wk_d = nc.declare_dram_parameter("Wk", [CK, C], F32, isOutput=False)
    wv_d = nc.declare_dram_parameter("Wv", [C, C], F32, isOutput=False)
    bq_d = nc.declare_dram_parameter("bq", [CK], F32, isOutput=False)
    bk_d = nc.declare_dram_parameter("bk", [CK], F32, isOutput=False)
    bv_d = nc.declare_dram_parameter("bv", [C], F32, isOutput=False)
    gam_d = nc.declare_dram_parameter("gamma", [1], F32, isOutput=False)
    out_d = nc.declare_dram_parameter("out", [C, NL], F32, isOutput=True)

    def bcast_ap(handle, parts, free):
        ap = handle[:]
        return bass.AP(tensor=ap.tensor, offset=ap.offset, ap=[[0, parts], [1, free]])

    consts = ctx.enter_context(tc.tile_pool(name="consts", bufs=1))
    persist = ctx.enter_context(tc.tile_pool(name="persist", bufs=1))

    ident = consts.tile([P, P], F32)
    make_identity(nc, ident)

    # Transposed weights (fp32, used as float32r): wq_t[p, ct, 32r+ck] =
    # Wq[ck, ct*128+p] for replica r in {0,1} (feeds the 2x-packed energy).
    wq_t = consts.tile([P, 2, 2 * CK], BF16)
    wk_t = consts.tile([P, 2, 2 * CK], BF16)
    # wv_t[p, ct, c] = Wv[c, ct*128+p]; column 256 stays 0
    wv_t = consts.tile([P, 2, C + 1], BF16)
    nc.vector.memset(wv_t[:], 0.0)
    # bv broadcast across partitions; column 256 = 1.0 (ones column of vT)
    bv_b = consts.tile([P, C + 1], F32)
    nc.vector.memset(bv_b[:], 1.0)
    nc.gpsimd.dma_start(out=bv_b[:, 0:C], in_=bcast_ap(bv_d, P, C))
    # per-partition bias columns, replicated for partitions 32..63
    bq_c = consts.tile([2 * CK, 1], F32)
    nc.sync.dma_start(out=bq_c[0:CK, :], in_=bq_d[:][:, None])
    nc.sync.dma_start(out=bq_c[CK:2 * CK, :], in_=bq_d[:][:, None])
    bk_c = consts.tile([2 * CK, 1], F32)
    nc.sync.dma_start(out=bk_c[0:CK, :], in_=bk_d[:][:, None])
    nc.sync.dma_start(out=bk_c[CK:2 * CK, :], in_=bk_d[:][:, None])
    gam = consts.tile([P, 1], F32)
    nc.gpsimd.dma_start(out=gam[:], in_=bcast_ap(gam_d, P, 1))

    # Residual features for this core's query half: [p, ct, i]
    # (loaded late — only needed by the finalize stage)
    feat_res = persist.tile([P, 2, NL], F32)

    # Projection outputs (persist across phases); partitions 32..63 hold a
    # replica of partitions 0..31 (for the 2x-packed energy matmuls).
    q_rep = persist.tile([P, NL], BF16)         # q[ck, i] x2
    k_rep = persist.tile([P, N], BF16)          # k[ck, j] x2
    vT_sb = persist.tile([P, NJT, C + 1], BF16)  # vT[j%128, jt, c] (+ones col)
    out_acc = persist.tile([P, NIT, C + 1], F32)

    # ---- Phase A: load fp32 inputs, project q/k/vT (float32r matmuls) ----
    with tc.tile_pool(name="loads", bufs=1) as loads, \
         tc.tile_pool(name="psA", bufs=2, space="PSUM") as psA:

        wq_raw = loads.tile([CK, C], F32)
        nc.sync.dma_start(out=wq_raw[:], in_=wq_d[:, :])
        wk_raw = loads.tile([CK, C], F32)
        nc.sync.dma_start(out=wk_raw[:], in_=wk_d[:, :])
        wv_raw = loads.tile([P, 2, C], F32)
        for cb in range(2):
            nc.sync.dma_start(out=wv_raw[:, cb, :], in_=wv_d[ts(cb, P), :])

        # bf16 inputs via casting SWDGE DMAs, chunked into 512-column
        # tiles so projection matmuls start as soon as their chunk lands
        cond_c = []
        for icc in range(NL // 512):
            t = loads.tile([P, 2, 512], BF16, tag=f"cond{icc}")
            for ct in range(2):
                nc.gpsimd.dma_start(out=t[:, ct, :],
                                    in_=cond_d[ts(ct, P), ts(icc, 512)])
            cond_c.append(t)
        feat_c = []
        for ncc in range(N // 512):
            t = loads.tile([P, 2, 512], BF16, tag=f"feat{ncc}")
            for ct in range(2):
                nc.gpsimd.dma_start(out=t[:, ct, :],
                                    in_=feat_d[ts(ct, P), ts(ncc, 512)])
            feat_c.append(t)

        # Weight transposes via PE; copy each psum twice to build replicas
        for ct in range(2):
            ps = psA.tile([P, CK], F32, tag="proj")
            nc.tensor.transpose(ps[:], wq_raw[:, ts(ct, P)], ident[0:CK, 0:CK])
            nc.vector.tensor_copy(wq_t[:, ct, 0:CK], ps[:])
            nc.vector.tensor_copy(wq_t[:, ct, CK:2 * CK], ps[:])
            ps = psA.tile([P, CK], F32, tag="proj")
            nc.tensor.transpose(ps[:], wk_raw[:, ts(ct, P)], ident[0:CK, 0:CK])
            nc.vector.tensor_copy(wk_t[:, ct, 0:CK], ps[:])
            nc.vector.tensor_copy(wk_t[:, ct, CK:2 * CK], ps[:])
        for cb in range(2):
            for ct in range(2):
                ps = psA.tile([P, P], F32, tag="proj")
                nc.tensor.transpose(ps[:], wv_raw[:, cb, ts(ct, P)], ident[:])
                nc.vector.tensor_copy(wv_t[:, ct, ts(cb, P)], ps[:])

        # q[ck, i] = sum_c Wq[ck, c] cond[c, i]  (+bq on the PSUM->SBUF copy)
        q_ps = psA.tile([P, NL], F32, tag="proj")
        for icc in range(NL // 512):
            for ct in range(2):
                nc.tensor.matmul(
                    q_ps[0:2 * CK, ts(icc, 512)], wq_t[:, ct, :],
                    cond_c[icc][:, ct, :],
                    start=(ct == 0), stop=(ct == 1))
        nc.vector.tensor_scalar(q_rep[0:2 * CK, :], q_ps[0:2 * CK, :],
                                bq_c[:], None, op0=mybir.AluOpType.add)

        # k[ck, j], per n-half
        for kh in range(2):
            k_ps = psA.tile([P, NL], F32, tag="proj")
            for ncc in range(NL // 512):
                for ct in range(2):
                    nc.tensor.matmul(
                        k_ps[0:2 * CK, ts(ncc, 512)], wk_t[:, ct, :],
                        feat_c[kh * 4 + ncc][:, ct, :],
                        start=(ct == 0), stop=(ct == 1))
            nc.vector.tensor_scalar(k_rep[0:2 * CK, ts(kh, NL)],
                                    k_ps[0:2 * CK, :], bk_c[:], None,
                                    op0=mybir.AluOpType.add)

        # vT[j, c] = sum_cf feat[cf, j] Wv[c, cf]  (+bv, +ones column)
        for jt in range(NJT):
            v_ps = psA.tile([P, C + 1], F32, tag="proj")
            ch, jl = divmod(jt, 4)
            for ct in range(2):
                nc.tensor.matmul(
                    v_ps[:], feat_c[ch][:, ct, ts(jl, P)],
                    wv_t[:, ct, :],
                    start=(ct == 0), stop=(ct == 1))
            nc.vector.tensor_tensor(vT_sb[:, jt, :], v_ps[:], bv_b[:],
                                    op=mybir.AluOpType.add)

    # ---- Phase B: energy -> exp -> PV (grouped), finalize per query tile --
    with tc.tile_pool(name="attn", bufs=2) as attnp, \
         tc.tile_pool(name="fin", bufs=3) as finp, \
         tc.tile_pool(name="stage", bufs=2) as stagep, \
         tc.tile_pool(name="eps", bufs=2, space="PSUM") as epsp, \
         tc.tile_pool(name="pvps", bufs=2, space="PSUM") as pvpsp, \
         tc.tile_pool(name="tpps", bufs=2, space="PSUM") as tppsp:

        stage_tiles = {}

        def finalize(it):
            # out[i, c] /= out[i, 256]; transpose to [c, i]; residual+gamma
            rcp = finp.tile([P, 1], F32, tag="rcp")
            nc.vector.reciprocal(rcp[:], out_acc[:, it, C:C + 1])
            on = finp.tile([P, C], F32, tag="on")
            nc.vector.tensor_scalar(on[:], out_acc[:, it, 0:C], rcp[:], None,
                                    op0=mybir.AluOpType.mult)
            qt, sl = divmod(it, 4)
            if sl == 0:
                st_tile = stagep.tile([P, 2, 512], F32, tag="stage")
                stage_tiles[qt] = st_tile
            st = stage_tiles[qt]
            for ct in range(2):
                tp = tppsp.tile([P, P], F32)
                nc.tensor.transpose(tp[:], on[:, ts(ct, P)], ident[:])
                nc.vector.scalar_tensor_tensor(
                    st[:, ct, ts(sl, P)], tp[:], gam[:],
                    feat_res[:, ct, ts(it, P)],
                    op0=mybir.AluOpType.mult, op1=mybir.AluOpType.add)
            if sl == 3:
                for ct in range(2):
                    nc.sync.dma_start(out=out_d[ts(ct, P), ts(qt, 512)],
                                      in_=st[:, ct, :])

        for ct in range(2):
            nc.sync.dma_start(out=feat_res[:, ct, :], in_=fres_d[ts(ct, P), :])

        def energy_unit(attnT, g, step):
            jl, ih = divmod(step, 2)
            jt = g * GJ + jl
            e_ps = epsp.tile([P, 1024], F32, name="e_ps")
            # 2x-packed: replicas on partitions 0..31 / 32..63 feed
            # disjoint PE row-groups, running concurrently
            nc.tensor.matmul(
                e_ps[:, 0:512],
                k_rep[0:CK, ts(jt, P)],
                q_rep[0:CK, ih * 1024:ih * 1024 + 512],
                start=True, stop=True, tile_position=(0, 0))
            nc.tensor.matmul(
                e_ps[:, 512:1024],
                k_rep[CK:2 * CK, ts(jt, P)],
                q_rep[CK:2 * CK, ih * 1024 + 512:ih * 1024 + 1024],
                start=True, stop=True, tile_position=(32, 0))
            nc.scalar.activation(
                attnT[:, jl, ts(ih, 1024)], e_ps[:],
                mybir.ActivationFunctionType.Exp)

        def pv_unit(attnT, g, it):
            pv = pvpsp.tile([P, C + 1], F32, name="pv")
            for jl in range(GJ):
                nc.tensor.matmul(
                    pv[:], attnT[:, jl, ts(it, P)], vT_sb[:, g * GJ + jl, :],
                    start=(jl == 0), stop=(jl == GJ - 1))
            if g == 0:
                nc.vector.tensor_copy(out_acc[:, it, :], pv[:])
            else:
                nc.vector.tensor_tensor(out_acc[:, it, :], pv[:],
                                        out_acc[:, it, :],
                                        op=mybir.AluOpType.add)
                if g == NG - 1:
                    finalize(it)

        # Software pipeline: interleave group g's energy/exp units with
        # group g-1's PV units so the PE always has dense matmul work
        # (prevents HAM re-throttling during the ACT-bound energy phase).
        attnTs = {}
        for g in range(NG + 1):
            if g < NG:
                attnT_t = attnp.tile([P, GJ, NL], BF16, name="attnT")
                attnTs[g] = attnT_t
            for step in range(16):
                if g < NG:
                    energy_unit(attnTs[g], g, step)
                if g > 0:
                    pv_unit(attnTs[g - 1], g - 1, step)


def _split_ctrl_waits(nc, cap=1):
    """Walrus in this image allows only ONE sync-wait command per
    instruction; Tile emits several on phase-boundary instructions (and one
    per live semaphore on the kernel-tail drain). Splitting the excess waits
    onto preceding same-engine NoOps is semantically identical (engine
    sequencers execute in order, so waiting on A then B == waiting on both)."""
    for fn in nc.m.functions:
        for bb in fn.blocks:
            insts = bb.instructions
            out = []
            changed = False
            for ins in insts:
                si = ins.sync_info
                if si is not None and si.on_wait and len(si.on_wait) > cap:
                    waits = list(si.on_wait)
                    for i, w in enumerate(waits[:-cap]):
                        nop = mybir.InstNoOp(
                            name=f"{ins.name}-w{i}",
                            engine=ins.engine,
                            ins=[], outs=[],
                            sync_info=mybir.SyncInfo(on_wait=[w], on_update=[]),
                        )
                        if hasattr(nc, "register_instruction"):
                            nc.register_instruction(nop, overwrite=True)
                        out.append(nop)
                    ins.sync_info = mybir.SyncInfo(
                        on_wait=waits[-cap:], on_update=list(si.on_update))
                    changed = True
                out.append(ins)
            if changed:
                insts[:] = out


def build_nc():
    nc = bass.Bass()
    with tile.TileContext(nc) as tc, ExitStack() as ctx:
        _emit(tc, ctx)
    _split_ctrl_waits(nc)
    return nc


def make_in_maps(features, conditions, Wq, bq, Wk, bk, Wv, bv, gamma):
    feat = np.ascontiguousarray(np.asarray(features, np.float32).reshape(B, C, N))
    cond = np.ascontiguousarray(np.asarray(conditions, np.float32).reshape(B, C, N))
    wq = np.ascontiguousarray(np.asarray(Wq, np.float32))
    wk = np.ascontiguousarray(np.asarray(Wk, np.float32))
    wv = np.ascontiguousarray(np.asarray(Wv, np.float32))
    bq_ = np.ascontiguousarray(np.asarray(bq, np.float32))
    bk_ = np.ascontiguousarray(np.asarray(bk, np.float32))
    bv_ = np.ascontiguousarray(np.asarray(bv, np.float32))
    gam_ = np.ascontiguousarray(np.asarray(gamma, np.float32).reshape(1))
    in_maps = []
    for core in range(NCORES):
        b, h = divmod(core, 2)
        n0 = h * NL
        in_maps.append({
            "feat": feat[b],
            "cond": np.ascontiguousarray(cond[b][:, n0:n0 + NL]),
            "fres": np.ascontiguousarray(feat[b][:, n0:n0 + NL]),
            "Wq": wq, "Wk": wk, "Wv": wv,
            "bq": bq_, "bk": bk_, "bv": bv_, "gamma": gam_,
        })
    return in_maps


def kernel(features, conditions, Wq, bq, Wk, bk, Wv, bv, gamma):
    global LAST_EXEC_TIME_NS, LAST_TRACE
    in_maps = make_in_maps(features, conditions, Wq, bq, Wk, bk, Wv, bv, gamma)
    nc = build_nc()
    trace = os.environ.get("BASS_KERNEL_TRACE", "0") == "1"
    res = run_bass_kernel_spmd(nc, in_maps, list(range(NCORES)), trace=trace)
    LAST_EXEC_TIME_NS = res.exec_time_ns
    LAST_TRACE = res.instructions_and_trace
    out = np.empty((B, C, N), np.float32)
    for core in range(NCORES):
        b, h = divmod(core, 2)
        out[b][:, h * NL:(h + 1) * NL] = res.results[core]["out"]
    return out.reshape(B, C, H, W)


# revision 10
# speedup vs baseline: 1.6270x; 1.1717x over previous
"""Trainium2 Bass kernel for ConditionCrossAttention2D.

Reference computation (per batch item b, with n = H*W spatial positions):
    q = Wq @ cond + bq            # [Ck, n] -> used as q[n, Ck]
    k = Wk @ feat + bk            # [Ck, n]
    v = Wv @ feat + bv            # [C, n]
    energy[i, j] = sum_ck q[ck, i] * k[ck, j]
    attn = softmax_j(energy)
    out[c, i] = sum_j v[c, j] * attn[i, j]
    result = gamma * out + feat

Sharding: 8 cores = (batch b in 0..3) x (query-half h in 0..1). Each core
computes the full [2048 x 4096] attention for its query half — no
cross-core communication.

Per-core layout choices:
  - energy is computed TRANSPOSED: e_T[j, i] (keys on partitions). The
    exp'd tile attnT[j, i] is then directly the stationary operand (lhsT)
    of the PV matmul out[i, c] = sum_j attnT[j, i] * vT[j, c]. No
    transposes in the inner loop.
  - softmax denominators come for free from an appended ones-column in
    vT (vT[j, 256] = 1), so out_psum[i, 256] = sum_j exp(energy[i, j]).
  - softmax max-subtraction is skipped: energies here are O(1) (weights
    are 0.02-scaled), exp is computed in fp32 — mathematically identical
    to the max-shifted softmax.
  - inputs are cast fp32->bf16 inside the load DMAs (SWDGE casting
    path), so no on-chip cast pass is needed.
  - the K=32 energy matmuls are packed 2x into disjoint PE row-groups
    via tile_position, with q/k replicated to partitions 32..63.
  - energy/PV matmuls use bf16 operands (fp32 PSUM accumulation);
    softmax statistics and the output path stay fp32.
"""

import os
from contextlib import ExitStack

import numpy as np

import concourse.bass as bass
import concourse.tile as tile
from concourse import mybir
from concourse.bass_utils import run_bass_kernel_spmd
from concourse.masks import make_identity

B, C, CK, H, W = 4, 256, 32, 64, 64
N = H * W            # 4096 spatial positions
NCORES = 8
NL = N // 2          # 2048 queries per core
P = 128
NJT = N // P         # 32 key tiles
NIT = NL // P        # 16 query tiles per core
GJ = 8               # key tiles per group
NG = NJT // GJ       # 4 groups
F32 = mybir.dt.float32
F32R = mybir.dt.float32r
BF16 = mybir.dt.bfloat16

LAST_EXEC_TIME_NS = None
LAST_TRACE = None

ts = bass.ts


def _emit(tc, ctx):
    nc = tc.nc

    feat_d = nc.declare_dram_parameter("feat", [C, N], F32, isOutput=False)
    cond_d = nc.declare_dram_parameter("cond", [C, NL], F32, isOutput=False)
    fres_d = nc.declare_dram_parameter("fres", [C, NL], F32, isOutput=False)
    wq_d = nc.declare_dram_parameter("Wq", [CK, C], F32, isOutput=False)
    wk_d = nc.declare_dram_parameter("Wk", [CK, C], F32, isOutput=False)
    wv_d = nc.declare_dram_parameter("Wv", [C, C], F32, isOutput=False)
    bq_d = nc.declare_dram_parameter("bq", [CK], F32, isOutput=False)
    bk_d = nc.declare_dram_parameter("bk", [CK], F32, isOutput=False)
    bv_d = nc.declare_dram_parameter("bv", [C], F32, isOutput=False)
    gam_d = nc.declare_dram_parameter("gamma", [1], F32, isOutput=False)
    out_d = nc.declare_dram_parameter("out", [C, NL], F32, isOutput=True)

    def bcast_ap(handle, parts, free):
        ap = handle[:]
        return bass.AP(tensor=ap.tensor, offset=ap.offset, ap=[[0, parts], [1, free]])

    consts = ctx.enter_context(tc.tile_pool(name="consts", bufs=1))
    persist = ctx.enter_context(tc.tile_pool(name="persist", bufs=1))
    loads = ctx.enter_context(tc.tile_pool(name="loads", bufs=1))
    attnp = ctx.enter_context(tc.tile_pool(name="attn", bufs=2))
    finp = ctx.enter_context(tc.tile_pool(name="fin", bufs=3))
    stagep = ctx.enter_context(tc.tile_pool(name="stage", bufs=2))
    # All PSUM, statically partitioned: 2x[*,1024] (4 banks) + 2x[128,257]
    # (2 banks) + 2x[128,128] (2 banks) = 8 banks.
    bigp = ctx.enter_context(tc.tile_pool(name="bigps", bufs=2, space="PSUM"))
    pvp = ctx.enter_context(tc.tile_pool(name="pvps", bufs=2, space="PSUM"))
    tpp = ctx.enter_context(tc.tile_pool(name="tpps", bufs=2, space="PSUM"))

    ident = consts.tile([P, P], F32)
    make_identity(nc, ident)

    # Transposed weights (bf16): wq_t[p, ct, 32r+ck] = Wq[ck, ct*128+p]
    # for replica r in {0,1} (feeds the 2x-packed energy matmuls).
    wq_t = consts.tile([P, 2, 2 * CK], BF16)
    wk_t = consts.tile([P, 2, 2 * CK], BF16)
    # wv_t[p, ct, c] = Wv[c, ct*128+p]; column 256 stays 0
    wv_t = consts.tile([P, 2, C + 1], BF16)
    nc.vector.memset(wv_t[:], 0.0)
    # bv broadcast across partitions; column 256 = 1.0 (ones column of vT)
    bv_b = consts.tile([P, C + 1], F32)
    nc.vector.memset(bv_b[:], 1.0)
    nc.gpsimd.dma_start(out=bv_b[:, 0:C], in_=bcast_ap(bv_d, P, C))
    # per-partition bias columns, replicated for partitions 32..63
    bq_c = consts.tile([2 * CK, 1], F32)
    nc.sync.dma_start(out=bq_c[0:CK, :], in_=bq_d[:][:, None])
    nc.sync.dma_start(out=bq_c[CK:2 * CK, :], in_=bq_d[:][:, None])
    bk_c = consts.tile([2 * CK, 1], F32)
    nc.sync.dma_start(out=bk_c[0:CK, :], in_=bk_d[:][:, None])
    nc.sync.dma_start(out=bk_c[CK:2 * CK, :], in_=bk_d[:][:, None])
    gam = consts.tile([P, 1], F32)
    nc.gpsimd.dma_start(out=gam[:], in_=bcast_ap(gam_d, P, 1))

    # Residual features for this core's query half (only needed by the
    # finalize stage; loaded after the compute-critical inputs).
    feat_res = persist.tile([P, 2, NL], F32)

    # Projection outputs; partitions 32..63 hold a replica of partitions
    # 0..31 (for the 2x-packed energy matmuls).
    q_rep = persist.tile([P, NL], BF16)         # q[ck, i] x2
    k_rep = persist.tile([P, N], BF16)          # k[ck, j] x2
    vT_sb = persist.tile([P, NJT, C + 1], BF16)  # vT[j%128, jt, c] (+ones col)
    out_acc = persist.tile([P, NIT, C + 1], F32)

    # ---- loads ----
    wq_raw = loads.tile([CK, C], F32)
    nc.sync.dma_start(out=wq_raw[:], in_=wq_d[:, :])
    wk_raw = loads.tile([CK, C], F32)
    nc.sync.dma_start(out=wk_raw[:], in_=wk_d[:, :])
    wv_raw = loads.tile([P, 2, C], F32)
    for cb in range(2):
        nc.sync.dma_start(out=wv_raw[:, cb, :], in_=wv_d[ts(cb, P), :])

    # bf16 inputs via casting SWDGE DMAs, chunked into 512-column tiles so
    # projection matmuls start as soon as their chunk lands
    cond_c = []
    for icc in range(NL // 512):
        t = loads.tile([P, 2, 512], BF16, tag=f"cond{icc}")
        for ct in range(2):
            nc.gpsimd.dma_start(out=t[:, ct, :],
                                in_=cond_d[ts(ct, P), ts(icc, 512)])
        cond_c.append(t)
    feat_c = []
    for ncc in range(N // 512):
        t = loads.tile([P, 2, 512], BF16, tag=f"feat{ncc}")
        for ct in range(2):
            nc.gpsimd.dma_start(out=t[:, ct, :],
                                in_=feat_d[ts(ct, P), ts(ncc, 512)])
        feat_c.append(t)

    # ---- weight transposes via PE; copy psum twice to build replicas ----
    for ct in range(2):
        ps = tpp.tile([P, CK], F32, tag="tp")
        nc.tensor.transpose(ps[:], wq_raw[:, ts(ct, P)], ident[0:CK, 0:CK])
        nc.vector.tensor_copy(wq_t[:, ct, 0:CK], ps[:])
        nc.vector.tensor_copy(wq_t[:, ct, CK:2 * CK], ps[:])
        ps = tpp.tile([P, CK], F32, tag="tp")
        nc.tensor.transpose(ps[:], wk_raw[:, ts(ct, P)], ident[0:CK, 0:CK])
        nc.vector.tensor_copy(wk_t[:, ct, 0:CK], ps[:])
        nc.vector.tensor_copy(wk_t[:, ct, CK:2 * CK], ps[:])
    for cb in range(2):
        for ct in range(2):
            ps = tpp.tile([P, P], F32, tag="tp")
            nc.tensor.transpose(ps[:], wv_raw[:, cb, ts(ct, P)], ident[:])
            nc.vector.tensor_copy(wv_t[:, ct, ts(cb, P)], ps[:])

    # ---- q and k projections, in [64, 1024] psum halves ----
    def q_half(hh):
        q_ps = bigp.tile([2 * CK, 1024], F32, tag="big", name="q_ps")
        for sc in range(2):
            icc = hh * 2 + sc
            for ct in range(2):
                nc.tensor.matmul(
                    q_ps[:, ts(sc, 512)], wq_t[:, ct, :],
                    cond_c[icc][:, ct, :],
                    start=(ct == 0), stop=(ct == 1))
        nc.vector.tensor_scalar(q_rep[0:2 * CK, ts(hh, 1024)], q_ps[:],
                                bq_c[:], None, op0=mybir.AluOpType.add)

    def k_half(hh):
        k_ps = bigp.tile([2 * CK, 1024], F32, tag="big", name="k_ps")
        for sc in range(2):
            ncc = hh * 2 + sc
            for ct in range(2):
                nc.tensor.matmul(
                    k_ps[:, ts(sc, 512)], wk_t[:, ct, :],
                    feat_c[ncc][:, ct, :],
                    start=(ct == 0), stop=(ct == 1))
        nc.vector.tensor_scalar(k_rep[0:2 * CK, ts(hh, 1024)], k_ps[:],
                                bk_c[:], None, op0=mybir.AluOpType.add)

    for hh in range(2):
        q_half(hh)
    for hh in range(2):
        k_half(hh)          # first n-half of k; halves 2-3 in the prologue

    # ---- vT projection unit ----
    def vt_unit(jt):
        v_ps = pvp.tile([P, C + 1], F32, tag="pv", name="v_ps")
        ch, jl = divmod(jt, 4)
        for ct in range(2):
            nc.tensor.matmul(
                v_ps[:], feat_c[ch][:, ct, ts(jl, P)],
                wv_t[:, ct, :],
                start=(ct == 0), stop=(ct == 1))
        nc.vector.tensor_tensor(vT_sb[:, jt, :], v_ps[:], bv_b[:],
                                op=mybir.AluOpType.add)

    # ---- phase B units ----
    stage_tiles = {}

    def finalize(it):
        # out[i, c] /= out[i, 256]; transpose to [c, i]; residual + gamma
        rcp = finp.tile([P, 1], F32, tag="rcp")
        nc.vector.reciprocal(rcp[:], out_acc[:, it, C:C + 1])
        on = finp.tile([P, C], F32, tag="on")
        nc.vector.tensor_scalar(on[:], out_acc[:, it, 0:C], rcp[:], None,
                                op0=mybir.AluOpType.mult)
        qt, sl = divmod(it, 4)
        if sl == 0:
            st_tile = stagep.tile([P, 2, 512], F32, tag="stage")
            stage_tiles[qt] = st_tile
        st = stage_tiles[qt]
        for ct in range(2):
            tp = tpp.tile([P, P], F32, tag="tp", name="tp")
            nc.tensor.transpose(tp[:], on[:, ts(ct, P)], ident[:])
            nc.vector.scalar_tensor_tensor(
                st[:, ct, ts(sl, P)], tp[:], gam[:],
                feat_res[:, ct, ts(it, P)],
                op0=mybir.AluOpType.mult, op1=mybir.AluOpType.add)
        if sl == 3:
            for ct in range(2):
                nc.sync.dma_start(out=out_d[ts(ct, P), ts(qt, 512)],
                                  in_=st[:, ct, :])

    def energy_unit(attnT, g, step):
        jl, ih = divmod(step, 2)
        jt = g * GJ + jl
        e_ps = bigp.tile([P, 1024], F32, tag="big", name="e_ps")
        # 2x-packed: replicas on partitions 0..31 / 32..63 feed disjoint
        # PE row-groups, running concurrently
        nc.tensor.matmul(
            e_ps[:, 0:512],
            k_rep[0:CK, ts(jt, P)],
            q_rep[0:CK, ih * 1024:ih * 1024 + 512],
            start=True, stop=True, tile_position=(0, 0))
        nc.tensor.matmul(
            e_ps[:, 512:1024],
            k_rep[CK:2 * CK, ts(jt, P)],
            q_rep[CK:2 * CK, ih * 1024 + 512:ih * 1024 + 1024],
            start=True, stop=True, tile_position=(32, 0))
        nc.scalar.activation(
            attnT[:, jl, ts(ih, 1024)], e_ps[:],
            mybir.ActivationFunctionType.Exp)

    def pv_unit(attnT, g, it):
        pv = pvp.tile([P, C + 1], F32, tag="pv", name="pv")
        for jl in range(GJ):
            nc.tensor.matmul(
                pv[:], attnT[:, jl, ts(it, P)], vT_sb[:, g * GJ + jl, :],
                start=(jl == 0), stop=(jl == GJ - 1))
        if g == 0:
            nc.vector.tensor_copy(out_acc[:, it, :], pv[:])
        else:
            nc.vector.tensor_tensor(out_acc[:, it, :], pv[:],
                                    out_acc[:, it, :],
                                    op=mybir.AluOpType.add)
            if g == NG - 1:
                finalize(it)

    for ct in range(2):
        nc.sync.dma_start(out=feat_res[:, ct, :], in_=fres_d[ts(ct, P), :])

    # Software pipeline: group g's energy/exp units interleave with group
    # g-1's PV units so the PE always has dense matmul work (prevents HAM
    # re-throttling during the ACT-bound energy phase). The prologue
    # (g == 0) interleaves the vT projections instead of PV work.
    attnTs = {}
    for g in range(NG + 1):
        if g < NG:
            attnT_t = attnp.tile([P, GJ, NL], BF16, name="attnT")
            attnTs[g] = attnT_t
        for step in range(16):
            if g < NG:
                energy_unit(attnTs[g], g, step)
            if g == 0:
                vt_unit(2 * step)
                vt_unit(2 * step + 1)
                if step in (8, 12):
                    k_half(2 + (step - 8) // 4)  # second n-half of k
            else:
                pv_unit(attnTs[g - 1], g - 1, step)


def _split_ctrl_waits(nc, cap=1):
    """Walrus in this image allows only ONE sync-wait command per
    instruction; Tile emits several on phase-boundary instructions (and one
    per live semaphore on the kernel-tail drain). Splitting the excess waits
    onto preceding same-engine NoOps is semantically identical (engine
    sequencers execute in order, so waiting on A then B == waiting on both)."""
    for fn in nc.m.functions:
        for bb in fn.blocks:
            insts = bb.instructions
            out = []
            changed = False
            for ins in insts:
                si = ins.sync_info
                if si is not None and si.on_wait and len(si.on_wait) > cap:
                    waits = list(si.on_wait)
                    for i, w in enumerate(waits[:-cap]):
                        nop = mybir.InstNoOp(
                            name=f"{ins.name}-w{i}",
                            engine=ins.engine,
                            ins=[], outs=[],
                            sync_info=mybir.SyncInfo(on_wait=[w], on_update=[]),
                        )
                        if hasattr(nc, "register_instruction"):
                            nc.register_instruction(nop, overwrite=True)
                        out.append(nop)
                    ins.sync_info = mybir.SyncInfo(
                        on_wait=waits[-cap:], on_update=list(si.on_update))
                    changed = True
                out.append(ins)
            if changed:
                insts[:] = out


def build_nc():
    nc = bass.Bass()
    with tile.TileContext(nc) as tc, ExitStack() as ctx:
        _emit(tc, ctx)
    _split_ctrl_waits(nc)
    return nc


def make_in_maps(features, conditions, Wq, bq, Wk, bk, Wv, bv, gamma):
    feat = np.ascontiguousarray(np.asarray(features, np.float32).reshape(B, C, N))
    cond = np.ascontiguousarray(np.asarray(conditions, np.float32).reshape(B, C, N))
    wq = np.ascontiguousarray(np.asarray(Wq, np.float32))
    wk = np.ascontiguousarray(np.asarray(Wk, np.float32))
    wv = np.ascontiguousarray(np.asarray(Wv, np.float32))
    bq_ = np.ascontiguousarray(np.asarray(bq, np.float32))
    bk_ = np.ascontiguousarray(np.asarray(bk, np.float32))
    bv_ = np.ascontiguousarray(np.asarray(bv, np.float32))
    gam_ = np.ascontiguousarray(np.asarray(gamma, np.float32).reshape(1))
    in_maps = []
    for core in range(NCORES):
        b, h = divmod(core, 2)
        n0 = h * NL
        in_maps.append({
            "feat": feat[b],
            "cond": np.ascontiguousarray(cond[b][:, n0:n0 + NL]),
            "fres": np.ascontiguousarray(feat[b][:, n0:n0 + NL]),
            "Wq": wq, "Wk": wk, "Wv": wv,
            "bq": bq_, "bk": bk_, "bv": bv_, "gamma": gam_,
        })
    return in_maps


def kernel(features, conditions, Wq, bq, Wk, bk, Wv, bv, gamma):
    global LAST_EXEC_TIME_NS, LAST_TRACE
    in_maps = make_in_maps(features, conditions, Wq, bq, Wk, bk, Wv, bv, gamma)
    nc = build_nc()
    trace = os.environ.get("BASS_KERNEL_TRACE", "0") == "1"
    res = run_bass_kernel_spmd(nc, in_maps, list(range(NCORES)), trace=trace)
    LAST_EXEC_TIME_NS = res.exec_time_ns
    LAST_TRACE = res.instructions_and_trace
    out = np.empty((B, C, N), np.float32)
    for core in range(NCORES):
        b, h = divmod(core, 2)
        out[b][:, h * NL:(h + 1) * NL] = res.results[core]["out"]
    return out.reshape(B, C, H, W)
